# revision 14
# baseline (speedup 1.0000x reference)
"""Trainium2 Bass kernel for the DAM attention module.

Math (per batch b):
  pq = wq . q + bq            [N]        (N = L*H*W = 8192)
  pk = wk . k + bk            [N]
  energy[m,n] = pq[m]*pk[n]   (rank-1 outer product -> never materialized in HBM)
  attn = softmax_n(energy)
  pv = wv . cat(v,q,k) + bv   [C,N]
  out[c,m] = sum_n pv[c,n]*attn[m,n];  return gamma*out + v

Kernel strategy (8 cores = 2 batches x 4 query-blocks of 2048):
  - E^T tiles [128(n) x 2048(m)] are generated on ScalarE in ONE activation op
    each: exp(scale*in) with per-partition scale = pk column, in = broadcast pq.
  - TensorE contracts pv' (pv with an appended ones-row) against E^T tiles,
    accumulating [65 x 512] PSUM blocks over 64 n-tiles. Row 64 of the result
    is the softmax denominator Z (ones-row trick) -> no separate reduction.
  - Softmax max-subtraction is skipped: |energy| <= ~14, exp() is exact in f32.
  - Epilogue: out = num * (gamma/Z) + v via a K=1 broadcast matmul + DVE.
"""

import numpy as np

import concourse.bacc as bacc
import concourse.bass as bass
import concourse.tile as tile
from concourse import mybir
from concourse.bass_utils import run_bass_kernel_spmd

F32 = mybir.dt.float32
F32R = mybir.dt.float32r
BF16 = mybir.dt.bfloat16

B, C, L, H, W = 2, 64, 8, 32, 32
N = L * H * W            # 8192
HALF = N // 2            # 4096
M_PER_CORE = N // 4      # 2048
NT = N // 128            # 64 n-tiles
MB = M_PER_CORE // 512   # 4 psum m-blocks per core

_CACHE = {}


def _build(reps=1):
    nc = bacc.Bacc()

    qh_d = nc.declare_dram_parameter("qh", [128, HALF], F32, isOutput=False)
    kh_d = nc.declare_dram_parameter("kh", [128, HALF], F32, isOutput=False)
    vh_d = nc.declare_dram_parameter("vh", [128, HALF], F32, isOutput=False)
    qm_d = nc.declare_dram_parameter("qm", [64, M_PER_CORE], F32, isOutput=False)
    vm_d = nc.declare_dram_parameter("vm", [64, M_PER_CORE], F32, isOutput=False)
    # packed params [128, 265]:
    #   col 0 = wk (x2), col 1 = bk (x128), cols 2:197 = wvT6 [128,3,65],
    #   col 197 rows 0:64 = wq, row 0 cols 198:263 = brow, 263 = bq, 264 = gx
    pp_d = nc.declare_dram_parameter("pp", [128, 265], F32, isOutput=False)
    out_d = nc.declare_dram_parameter("out", [64, M_PER_CORE], F32, isOutput=True)

    with tile.TileContext(nc) as tc:
        with (
            tc.tile_pool(name="singles", bufs=1) as singles,
            tc.tile_pool(name="et", bufs=3) as et_pool,
            tc.tile_pool(name="outp", bufs=2) as outp,
            tc.tile_pool(name="ps_out", bufs=1, space="PSUM") as ps_out,
            tc.tile_pool(name="ps_sc", bufs=2, space="PSUM") as ps_sc,
            tc.tile_pool(name="ps_pk", bufs=1, space="PSUM") as ps_pk,
            tc.tile_pool(name="ps_pv", bufs=1, space="PSUM") as ps_pv,
        ):
            # ---- load inputs (SWDGE: one semaphore per load) ----
            qh = singles.tile([128, HALF], F32)
            kh = singles.tile([128, HALF], F32)
            vh = singles.tile([128, HALF], F32)
            nc.gpsimd.dma_start(out=qh[:], in_=qh_d[:])
            nc.gpsimd.dma_start(out=kh[:], in_=kh_d[:])
            nc.gpsimd.dma_start(out=vh[:], in_=vh_d[:])
            qm = singles.tile([64, M_PER_CORE], F32)
            nc.gpsimd.dma_start(out=qm[:], in_=qm_d[:])
            vm = singles.tile([64, M_PER_CORE], F32)
            nc.gpsimd.dma_start(out=vm[:], in_=vm_d[:])
            pp = singles.tile([128, 265], F32)
            nc.gpsimd.dma_start(out=pp[:], in_=pp_d[:])
            wk = pp[:, 0:1]
            bk = pp[:, 1:2]
            wq = pp[0:64, 197:198]
            brow = pp[0:1, 198:263]
            bq = pp[0:1, 263:264]
            gx = pp[0:1, 264:265]
            ones = singles.tile([1, 128], F32)
            nc.vector.memset(ones[:], 1.0)

            # ---- pq row + broadcast to all 128 partitions ----
            pq_row = singles.tile([1, M_PER_CORE], F32)
            pq_b = singles.tile([128, M_PER_CORE], F32)
            for j in range(MB):
                ps = ps_sc.tile([1, 512], F32, tag="sc")
                nc.tensor.matmul(ps[:], wq[:], qm[:, j * 512:(j + 1) * 512])
                nc.scalar.activation(
                    out=pq_row[:, j * 512:(j + 1) * 512], in_=ps[:],
                    func=mybir.ActivationFunctionType.Identity, bias=bq[:],
                )
            for j in range(MB):
                ps = ps_sc.tile([128, 512], F32, tag="sc")
                nc.tensor.matmul(ps[:], ones[:], pq_row[0:1, j * 512:(j + 1) * 512])
                nc.vector.tensor_copy(pq_b[:, j * 512:(j + 1) * 512], ps[:])

            # ---- pk in [128(n-in-tile), 64(tile)] layout ----
            pk_ps = ps_pk.tile([128, NT], F32)
            for t in range(NT):
                hb = (t // 32) * 64
                c0 = (t % 32) * 128
                nc.tensor.matmul(
                    pk_ps[:, t:t + 1],
                    kh[hb:hb + 64, c0:c0 + 128],
                    wk[hb:hb + 64, :],
                )
            pk_sb = singles.tile([128, NT], F32)
            nc.scalar.activation(
                out=pk_sb[:], in_=pk_ps[:],
                func=mybir.ActivationFunctionType.Identity, bias=bk[:],
            )

            # ---- pv' in transposed layout [128(n), 65(c; col 64 = ones)] ----
            pvT = singles.tile([128, NT, 65], BF16)
            for g in range(NT // 4):           # 4 n-tiles per PSUM bank
                pv_ps = ps_pv.tile([128, 4, 65], F32)
                for u in range(4):
                    t = g * 4 + u
                    hb = (t // 32) * 64
                    c0 = (t % 32) * 128
                    for i, src in enumerate((vh, qh, kh)):
                        nc.tensor.matmul(
                            pv_ps[:, u, :],
                            src[hb:hb + 64, c0:c0 + 128],
                            pp[hb:hb + 64, 2 + 65 * i:2 + 65 * (i + 1)],
                            start=(i == 0), stop=False,
                        )
                    nc.tensor.matmul(
                        pv_ps[:, u, :], ones[:], brow[:],
                        start=False, stop=True,
                    )
                nc.vector.tensor_copy(pvT[:, g * 4:(g + 1) * 4, :], pv_ps[:])

            # ---- main loop: E^T tiles on ScalarE, contraction on TensorE ----
            num = [
                ps_out.tile([65, 512], F32, tag=f"o{j}", name=f"num{j}")
                for j in range(MB)
            ]
            for r in range(reps):
                for t in range(NT):
                    et = et_pool.tile([128, M_PER_CORE], BF16, name="et")
                    nc.scalar.activation(
                        out=et[:], in_=pq_b[:],
                        func=mybir.ActivationFunctionType.Exp,
                        scale=pk_sb[:, t:t + 1],
                    )
                    for j in range(MB):
                        nc.tensor.matmul(
                            num[j][:],
                            pvT[:, t, :],
                            et[:, j * 512:(j + 1) * 512],
                            start=(r == 0 and t == 0),
                            stop=(r == reps - 1 and t == NT - 1),
                        )

            # ---- epilogue: out = num * (gamma/Z) + v ----
            for j in range(MB):
                recip = outp.tile([1, 512], F32, tag="recip")
                nc.vector.reciprocal(recip[:], num[j][64:65, :])
                nc.vector.tensor_scalar_mul(recip[:], recip[:], gx[:])
                ps_b = ps_sc.tile([64, 512], F32, tag="sc")
                nc.tensor.matmul(ps_b[:], ones[0:1, 0:64], recip[:])
                rb = outp.tile([64, 512], F32, tag="rb")
                nc.scalar.copy(rb[:], ps_b[:])
                o = outp.tile([64, 512], F32, tag="o")
                nc.vector.tensor_mul(o[:], num[j][0:64, :], rb[:])
                nc.vector.tensor_add(o[:], o[:], vm[:, j * 512:(j + 1) * 512])
                nc.gpsimd.dma_start(out=out_d[:, j * 512:(j + 1) * 512], in_=o[:])

    nc.compile()
    return nc


def _prep_core_inputs(q2, k2, v2, wq, bq, wk, bk, wv, bv, gamma_x):
    """Host-side shard prep. q2/k2/v2: [B, C, N] float32."""
    wvT = np.ascontiguousarray(wv.T)                     # [192, 64]
    parts = wvT.reshape(3, 64, 64)
    wvp = np.zeros((3, 64, 65), np.float32)
    wvp[:, :, :64] = parts
    wvT6 = np.tile(wvp.transpose(1, 0, 2), (2, 1, 1))    # [128, 3, 65]

    pp = np.zeros((128, 265), np.float32)
    pp[:, 0] = np.tile(wk.astype(np.float32), 2)
    pp[:, 1] = bk.astype(np.float32)[0]
    pp[:, 2:197] = wvT6.reshape(128, 195)
    pp[0:64, 197] = wq.astype(np.float32)
    pp[0, 198:262] = bv.astype(np.float32)
    pp[0, 262] = 1.0
    pp[0, 263] = bq.astype(np.float32)[0]
    pp[0, 264] = gamma_x.astype(np.float32)[0]

    in_maps = []
    for core in range(8):
        b, jm = core // 4, core % 4
        m0 = jm * M_PER_CORE
        in_maps.append({
            "qh": np.ascontiguousarray(
                np.concatenate([q2[b][:, :HALF], q2[b][:, HALF:]], axis=0)),
            "kh": np.ascontiguousarray(
                np.concatenate([k2[b][:, :HALF], k2[b][:, HALF:]], axis=0)),
            "vh": np.ascontiguousarray(
                np.concatenate([v2[b][:, :HALF], v2[b][:, HALF:]], axis=0)),
            "qm": np.ascontiguousarray(q2[b][:, m0:m0 + M_PER_CORE]),
            "vm": np.ascontiguousarray(v2[b][:, m0:m0 + M_PER_CORE]),
            "pp": pp,
        })
    return in_maps


def kernel(q, k, v, wq, bq, wk, bk, wv, bv, gamma_x):
    q2 = np.asarray(q, np.float32).reshape(B, C, N)
    k2 = np.asarray(k, np.float32).reshape(B, C, N)
    v2 = np.asarray(v, np.float32).reshape(B, C, N)
    in_maps = _prep_core_inputs(
        q2, k2, v2,
        np.asarray(wq, np.float32), np.asarray(bq, np.float32),
        np.asarray(wk, np.float32), np.asarray(bk, np.float32),
        np.asarray(wv, np.float32), np.asarray(bv, np.float32),
        np.asarray(gamma_x, np.float32),
    )

    if "nc" not in _CACHE:
        _CACHE["nc"] = _build()
    res = run_bass_kernel_spmd(_CACHE["nc"], in_maps, list(range(8)))

    out = np.empty((B, C, N), np.float32)
    for core in range(8):
        b, jm = core // 4, core % 4
        m0 = jm * M_PER_CORE
        out[b, :, m0:m0 + M_PER_CORE] = res.results[core]["out"]
    return out.reshape(B, C, L, H, W)


# revision 15
# speedup vs baseline: 1.6509x; 1.6509x over previous
"""Trainium2 Bass kernel for the DAM attention module.

Math (per batch b):
  pq = wq . q + bq            [N]        (N = L*H*W = 8192)
  pk = wk . k + bk            [N]
  energy[m,n] = pq[m]*pk[n]   (rank-1 outer product -> never materialized in HBM)
  attn = softmax_n(energy)
  pv = wv . cat(v,q,k) + bv   [C,N]
  out[c,m] = sum_n pv[c,n]*attn[m,n];  return gamma*out + v

Kernel strategy (8 cores = 2 batches x 4 query-blocks of 2048):
  - E^T tiles [128(n) x 2048(m)] are generated on ScalarE in ONE activation op
    each: exp(scale*in) with per-partition scale = pk column, in = broadcast pq.
  - TensorE contracts pv' (pv with an appended ones-row) against E^T tiles,
    accumulating [65 x 512] PSUM blocks over 64 n-tiles. Row 64 of the result
    is the softmax denominator Z (ones-row trick) -> no separate reduction.
  - Softmax max-subtraction is skipped: |energy| <= ~14, exp() is exact in f32.
  - Epilogue: out = num * (gamma/Z) + v via a K=1 broadcast matmul + DVE.
"""

import numpy as np

import concourse.bacc as bacc
import concourse.bass as bass
import concourse.tile as tile
from concourse import mybir
from concourse.bass_utils import run_bass_kernel_spmd

F32 = mybir.dt.float32
F32R = mybir.dt.float32r
BF16 = mybir.dt.bfloat16

B, C, L, H, W = 2, 64, 8, 32, 32
N = L * H * W            # 8192
HALF = N // 2            # 4096
M_PER_CORE = N // 4      # 2048
NT = N // 128            # 64 n-tiles
MB = M_PER_CORE // 512   # 4 psum m-blocks per core

_CACHE = {}


def _build(reps=1):
    nc = bacc.Bacc()

    qh_d = nc.declare_dram_parameter("qh", [128, HALF], F32, isOutput=False)
    kh_d = nc.declare_dram_parameter("kh", [128, HALF], F32, isOutput=False)
    vh_d = nc.declare_dram_parameter("vh", [128, HALF], F32, isOutput=False)
    # packed params [128, 265]:
    #   col 0 = wk (x2), col 1 = bk (x128), cols 2:197 = wvT6 [128,3,65],
    #   col 197 rows 0:64 = wq, row 0 cols 198:263 = brow, 263 = bq, 264 = gx
    pp_d = nc.declare_dram_parameter("pp", [128, 265], F32, isOutput=False)
    out_d = nc.declare_dram_parameter("out", [64, M_PER_CORE], F32, isOutput=True)

    with tile.TileContext(nc) as tc:
        with (
            tc.tile_pool(name="singles", bufs=1) as singles,
            tc.tile_pool(name="et", bufs=3) as et_pool,
            tc.tile_pool(name="outp", bufs=2) as outp,
            tc.tile_pool(name="ps_out", bufs=1, space="PSUM") as ps_out,
            tc.tile_pool(name="ps_sc", bufs=2, space="PSUM") as ps_sc,
            tc.tile_pool(name="ps_pk", bufs=1, space="PSUM") as ps_pk,
            tc.tile_pool(name="ps_pv", bufs=1, space="PSUM") as ps_pv,
        ):
            # ---- load inputs (SWDGE: one semaphore per load) ----
            qh = singles.tile([128, HALF], F32)
            kh = singles.tile([128, HALF], F32)
            vh = singles.tile([128, HALF], F32)
            nc.gpsimd.dma_start(out=qh[:], in_=qh_d[:])
            nc.gpsimd.dma_start(out=kh[:], in_=kh_d[:])
            nc.gpsimd.dma_start(out=vh[:], in_=vh_d[:])
            # per-core n-axis is host-rolled so the query block is cols 0:2048
            qm = qh[0:64, 0:M_PER_CORE]
            vm = vh[0:64, 0:M_PER_CORE]
            pp = singles.tile([128, 265], F32)
            nc.gpsimd.dma_start(out=pp[:], in_=pp_d[:])
            wk = pp[:, 0:1]
            bk = pp[:, 1:2]
            wq = pp[0:64, 197:198]
            brow = pp[0:1, 198:263]
            bq = pp[0:1, 263:264]
            gx = pp[0:1, 264:265]
            ones = singles.tile([1, 128], F32)
            nc.vector.memset(ones[:], 1.0)

            # ---- pq row + broadcast to all 128 partitions ----
            pq_row = singles.tile([1, M_PER_CORE], F32)
            pq_b = singles.tile([128, M_PER_CORE], F32)
            for j in range(MB):
                ps = ps_sc.tile([1, 512], F32, tag="sc")
                nc.tensor.matmul(ps[:], wq[:], qm[:, j * 512:(j + 1) * 512])
                nc.scalar.activation(
                    out=pq_row[:, j * 512:(j + 1) * 512], in_=ps[:],
                    func=mybir.ActivationFunctionType.Identity, bias=bq[:],
                )
            for j in range(MB):
                ps = ps_sc.tile([128, 512], F32, tag="sc")
                nc.tensor.matmul(ps[:], ones[:], pq_row[0:1, j * 512:(j + 1) * 512])
                nc.vector.tensor_copy(pq_b[:, j * 512:(j + 1) * 512], ps[:])

            # ---- pk in [128(n-in-tile), 64(tile)] layout ----
            pk_ps = ps_pk.tile([128, NT], F32)
            for t in range(NT):
                hb = (t // 32) * 64
                c0 = (t % 32) * 128
                nc.tensor.matmul(
                    pk_ps[:, t:t + 1],
                    kh[hb:hb + 64, c0:c0 + 128],
                    wk[hb:hb + 64, :],
                )
            pk_sb = singles.tile([128, NT], F32)
            nc.scalar.activation(
                out=pk_sb[:], in_=pk_ps[:],
                func=mybir.ActivationFunctionType.Identity, bias=bk[:],
            )

            # ---- pv' in transposed layout [128(n), 65(c; col 64 = ones)] ----
            pvT = singles.tile([128, NT, 65], BF16)
            for g in range(NT // 4):           # 4 n-tiles per PSUM bank
                pv_ps = ps_pv.tile([128, 4, 65], F32)
                for u in range(4):
                    t = g * 4 + u
                    hb = (t // 32) * 64
                    c0 = (t % 32) * 128
                    for i, src in enumerate((vh, qh, kh)):
                        nc.tensor.matmul(
                            pv_ps[:, u, :],
                            src[hb:hb + 64, c0:c0 + 128],
                            pp[hb:hb + 64, 2 + 65 * i:2 + 65 * (i + 1)],
                            start=(i == 0), stop=False,
                        )
                    nc.tensor.matmul(
                        pv_ps[:, u, :], ones[:], brow[:],
                        start=False, stop=True,
                    )
                nc.vector.tensor_copy(pvT[:, g * 4:(g + 1) * 4, :], pv_ps[:])

            # ---- main loop: E^T tiles on ScalarE, contraction on TensorE ----
            num = [
                ps_out.tile([65, 512], F32, tag=f"o{j}", name=f"num{j}")
                for j in range(MB)
            ]
            for r in range(reps):
                for t in range(NT):
                    et = et_pool.tile([128, M_PER_CORE], BF16, name="et")
                    nc.scalar.activation(
                        out=et[:], in_=pq_b[:],
                        func=mybir.ActivationFunctionType.Exp,
                        scale=pk_sb[:, t:t + 1],
                    )
                    for j in range(MB):
                        nc.tensor.matmul(
                            num[j][:],
                            pvT[:, t, :],
                            et[:, j * 512:(j + 1) * 512],
                            start=(r == 0 and t == 0),
                            stop=(r == reps - 1 and t == NT - 1),
                        )

            # ---- epilogue: out = num * (gamma/Z) + v ----
            for j in range(MB):
                recip = outp.tile([1, 512], F32, tag="recip")
                nc.vector.reciprocal(recip[:], num[j][64:65, :])
                nc.vector.tensor_scalar_mul(recip[:], recip[:], gx[:])
                ps_b = ps_sc.tile([64, 512], F32, tag="sc")
                nc.tensor.matmul(ps_b[:], ones[0:1, 0:64], recip[:])
                rb = outp.tile([64, 512], F32, tag="rb")
                nc.scalar.copy(rb[:], ps_b[:])
                o = outp.tile([64, 512], F32, tag="o")
                nc.vector.tensor_mul(o[:], num[j][0:64, :], rb[:])
                nc.vector.tensor_add(o[:], o[:], vm[:, j * 512:(j + 1) * 512])
                nc.gpsimd.dma_start(out=out_d[:, j * 512:(j + 1) * 512], in_=o[:])

    nc.compile()
    return nc


def _prep_core_inputs(q2, k2, v2, wq, bq, wk, bk, wv, bv, gamma_x):
    """Host-side shard prep. q2/k2/v2: [B, C, N] float32."""
    wvT = np.ascontiguousarray(wv.T)                     # [192, 64]
    parts = wvT.reshape(3, 64, 64)
    wvp = np.zeros((3, 64, 65), np.float32)
    wvp[:, :, :64] = parts
    wvT6 = np.tile(wvp.transpose(1, 0, 2), (2, 1, 1))    # [128, 3, 65]

    pp = np.zeros((128, 265), np.float32)
    pp[:, 0] = np.tile(wk.astype(np.float32), 2)
    pp[:, 1] = bk.astype(np.float32)[0]
    pp[:, 2:197] = wvT6.reshape(128, 195)
    pp[0:64, 197] = wq.astype(np.float32)
    pp[0, 198:262] = bv.astype(np.float32)
    pp[0, 262] = 1.0
    pp[0, 263] = bq.astype(np.float32)[0]
    pp[0, 264] = gamma_x.astype(np.float32)[0]

    def halves(x):
        return np.ascontiguousarray(np.concatenate([x[:, :HALF], x[:, HALF:]], axis=0))

    in_maps = []
    for core in range(8):
        b, jm = core // 4, core % 4
        m0 = jm * M_PER_CORE
        # roll the n-axis so this core's query block is at columns 0:2048;
        # the contraction over n is order-agnostic as long as q/k/v share
        # the same permutation
        in_maps.append({
            "qh": halves(np.roll(q2[b], -m0, axis=1)),
            "kh": halves(np.roll(k2[b], -m0, axis=1)),
            "vh": halves(np.roll(v2[b], -m0, axis=1)),
            "pp": pp,
        })
    return in_maps


def kernel(q, k, v, wq, bq, wk, bk, wv, bv, gamma_x):
    q2 = np.asarray(q, np.float32).reshape(B, C, N)
    k2 = np.asarray(k, np.float32).reshape(B, C, N)
    v2 = np.asarray(v, np.float32).reshape(B, C, N)
    in_maps = _prep_core_inputs(
        q2, k2, v2,
        np.asarray(wq, np.float32), np.asarray(bq, np.float32),
        np.asarray(wk, np.float32), np.asarray(bk, np.float32),
        np.asarray(wv, np.float32), np.asarray(bv, np.float32),
        np.asarray(gamma_x, np.float32),
    )

    if "nc" not in _CACHE:
        _CACHE["nc"] = _build()
    res = run_bass_kernel_spmd(_CACHE["nc"], in_maps, list(range(8)))

    out = np.empty((B, C, N), np.float32)
    for core in range(8):
        b, jm = core // 4, core % 4
        m0 = jm * M_PER_CORE
        out[b, :, m0:m0 + M_PER_CORE] = res.results[core]["out"]
    return out.reshape(B, C, L, H, W)
